# revision 1
# baseline (speedup 1.0000x reference)
"""Triangular (symmetry-exploiting) contrastive-loss kernel for 8 NeuronCores.

s = xn @ xn.T is symmetric, so only cells (a, b) with a <= b of a 16x16 grid
of 512x512 cells need computing. Each computed off-diagonal cell contributes:
  - row sums for its 512 rows  (free-dim reduce: fused into ACT/DVE ops)
  - col sums for its 512 cols  (partition reduce: ones-vector matmul on PE)

Core c owns row-bands {c, 15-c}: 16-c + c+1 = 17 cells on every core. The
device program is core-uniform; per-core cell schedules are baked into the
host-prepped operand streams (xa = lhsT block, xb = rhs block per cell).
Host assembles global per-row sums from row/col partials, then ln + mean.

Per (cell, rb in 4):
  PE : 16 accumulating matmuls -> G = xn_rb @ xn_b.T in PSUM [128, 512]
  ACT: E = exp(G/tau) -> bf16, fused row-sum accum_out
  DVE: EM = (ycols == yown)*E, fused row-sum accum_out (S2S2D2 STT op)
  PE : colsum_E  += ones.T @ E   (accumulated over rb in PSUM [1, 512])
  PE : colsum_EM += ones.T @ EM
"""

import sys

import numpy as np
import ml_dtypes

if "/opt/trn_rl_repo" not in sys.path:
    sys.path.insert(0, "/opt/trn_rl_repo")

import concourse.bass as bass
import concourse.tile as tile
from concourse import bacc, mybir
from concourse.bass_utils import run_bass_kernel_spmd

TAU = 0.1
N, D = 8192, 2048
NCORES = 8
NB = 16                    # 512-row bands
BS = N // NB               # 512 band size
RBC = BS // 128            # 4 row-blocks per cell
KC = D // 128              # 16 contraction chunks
NCELL = NB + 1             # 17 cells per core
BF16 = mybir.dt.bfloat16
F32 = mybir.dt.float32


def core_cells(c):
    """Cell schedule for core c: [(a, b), ...] with a <= b, 17 cells.

    Both diagonal cells sit at k=0,1 on every core, so the (uniform) device
    program can statically skip their column-sum work (host discards it).
    """
    cells = [(c, c), (NB - 1 - c, NB - 1 - c)]
    cells += [(c, b) for b in range(c + 1, NB)]
    cells += [(NB - 1 - c, b) for b in range(NB - c, NB)]
    assert len(cells) == NCELL
    return cells

NDIAG = 2  # cells 0..NDIAG-1 are diagonal: no column-sum contribution


def build_bass():
    nc = bacc.Bacc(None, target_bir_lowering=False)

    # Per-cell operand streams (host baked the schedule into these):
    #   xa[k*128+p, kc*BS + rb*128 + m] = xn[a_k*BS + rb*128 + m, kc*128 + p]
    #   xb[k*128+p, kc*BS + jj]         = xn[b_k*BS + jj,          kc*128 + p]
    #   yc[k*128+p, jj]  = y[b_k*BS + jj]        (column labels, bcast over p)
    #   yo[p, k*RBC+rb]  = y[a_k*BS + rb*128+p]  (own row labels)
    xa = nc.dram_tensor("xa", [NCELL * 128, KC * BS], BF16, kind="ExternalInput")
    xb = nc.dram_tensor("xb", [NCELL * 128, KC * BS], BF16, kind="ExternalInput")
    yc = nc.dram_tensor("yc", [NCELL * 128, BS], BF16, kind="ExternalInput")
    yo = nc.dram_tensor("yo", [128, NCELL * RBC], BF16, kind="ExternalInput")
    rows_all = nc.dram_tensor("rows_all", [128, NCELL * RBC], F32, kind="ExternalOutput")
    rows_same = nc.dram_tensor("rows_same", [128, NCELL * RBC], F32, kind="ExternalOutput")
    cols_all = nc.dram_tensor("cols_all", [1, NCELL * BS], F32, kind="ExternalOutput")
    cols_same = nc.dram_tensor("cols_same", [1, NCELL * BS], F32, kind="ExternalOutput")

    with (
        tile.TileContext(nc) as tc,
        tc.tile_pool(name="xap", bufs=3) as xap,
        tc.tile_pool(name="xbp", bufs=3) as xbp,
        tc.tile_pool(name="ycp", bufs=3) as ycp,
        tc.tile_pool(name="res", bufs=1) as res,
        tc.tile_pool(name="ep", bufs=3) as ep,
        tc.tile_pool(name="ebp", bufs=3) as ebp,
        tc.tile_pool(name="emp", bufs=3) as emp,
        tc.tile_pool(name="psum", bufs=4, space="PSUM") as pp,
        tc.tile_pool(name="cpsum", bufs=2, space="PSUM") as cpp,
        tc.tile_pool(name="colst", bufs=4) as colst,
    ):
        yo_t = res.tile([128, NCELL * RBC], BF16)
        nc.sync.dma_start(out=yo_t[:], in_=yo[:])
        ones_t = res.tile([128, 1], BF16)
        nc.vector.memset(ones_t[:], 1.0)

        stage_all = res.tile([128, NCELL * RBC], F32)
        stage_same = res.tile([128, NCELL * RBC], F32)

        for k in range(NCELL):
            xa_t = xap.tile([128, KC * BS], BF16)
            nc.sync.dma_start(out=xa_t[:], in_=xa[k * 128 : (k + 1) * 128, :])
            xb_t = xbp.tile([128, KC * BS], BF16)
            nc.sync.dma_start(out=xb_t[:], in_=xb[k * 128 : (k + 1) * 128, :])
            yc_t = ycp.tile([128, BS], BF16)
            nc.sync.dma_start(out=yc_t[:], in_=yc[k * 128 : (k + 1) * 128, :])

            offdiag = k >= NDIAG
            if offdiag:
                cps_e = cpp.tile([1, BS], F32)
                cps_m = cpp.tile([1, BS], F32)
            for rb in range(RBC):
                ps = pp.tile([128, BS], F32)
                for kc in range(KC):
                    w = xa_t[:, kc * BS + rb * 128 : kc * BS + (rb + 1) * 128]
                    nc.tensor.matmul(
                        ps[:],
                        w,
                        xb_t[:, kc * BS : (kc + 1) * BS],
                        start=(kc == 0),
                        stop=(kc == KC - 1),
                    )
                slot = k * RBC + rb
                # e_t stays f32 so the (diag-dominated) row sums see no bf16
                # rounding; col sums (off-diag only) use a bf16 copy.
                e_t = ep.tile([128, BS], F32)
                nc.scalar.activation(
                    out=e_t[:],
                    in_=ps[:],
                    func=mybir.ActivationFunctionType.Exp,
                    scale=1.0 / TAU,
                    accum_out=stage_all[:, slot : slot + 1],
                )
                if offdiag:
                    e_b = ebp.tile([128, BS], BF16)
                    nc.vector.tensor_copy(out=e_b[:], in_=e_t[:])
                em_t = emp.tile([128, BS], BF16)
                nc.vector.scalar_tensor_tensor(
                    out=em_t[:],
                    in0=yc_t[:],
                    scalar=yo_t[:, slot : slot + 1],
                    in1=e_t[:],
                    op0=mybir.AluOpType.is_equal,
                    op1=mybir.AluOpType.mult,
                    accum_out=stage_same[:, slot : slot + 1],
                )
                # Column sums via ones-vector matmuls, accumulated over rb.
                if offdiag:
                    nc.tensor.matmul(
                        cps_e[:], ones_t[:], e_b[:], start=(rb == 0), stop=(rb == RBC - 1)
                    )
                    nc.tensor.matmul(
                        cps_m[:], ones_t[:], em_t[:], start=(rb == 0), stop=(rb == RBC - 1)
                    )
            if offdiag:
                # Copy col partials PSUM->SBUF (ACT is closest to PSUM), then
                # DMA straight to DRAM per cell -- no big staging tile.
                ca = colst.tile([1, BS], F32)
                nc.scalar.copy(out=ca[:], in_=cps_e[:])
                nc.sync.dma_start(out=cols_all[:, k * BS : (k + 1) * BS], in_=ca[:])
                cm = colst.tile([1, BS], F32)
                nc.scalar.copy(out=cm[:], in_=cps_m[:])
                nc.sync.dma_start(out=cols_same[:, k * BS : (k + 1) * BS], in_=cm[:])

        nc.sync.dma_start(out=rows_all[:], in_=stage_all[:])
        nc.sync.dma_start(out=rows_same[:], in_=stage_same[:])

    nc.compile()
    return nc


_CACHE: dict = {}


def _get_nc():
    if "nc" not in _CACHE:
        _CACHE["nc"] = build_bass()
    return _CACHE["nc"]


def _prep_inputs(x, y):
    x = np.ascontiguousarray(np.asarray(x, dtype=np.float32))
    y = np.asarray(y).astype(np.int32)
    xn = x / np.linalg.norm(x, axis=1, keepdims=True)
    xnb = xn.astype(ml_dtypes.bfloat16)
    ybf = y.astype(ml_dtypes.bfloat16)

    # Per-band prepped blocks, shared across cores:
    #   blkT[a][p, kc, jj] = xn[a*BS + jj, kc*128 + p]   -> [128, KC*BS]
    blkT = [
        np.ascontiguousarray(
            xnb[a * BS : (a + 1) * BS]          # [BS, D]
            .reshape(BS, KC, 128)               # [jj, kc, p]
            .transpose(2, 1, 0)                 # [p, kc, jj]
            .reshape(128, KC * BS)
        )
        for a in range(NB)
    ]
    #   blkA[a][p, kc, rb, m] = xn[a*BS + rb*128 + m, kc*128 + p]
    blkA = [
        np.ascontiguousarray(
            xnb[a * BS : (a + 1) * BS]          # [BS, D]
            .reshape(RBC, 128, KC, 128)         # [rb, m, kc, p]
            .transpose(3, 2, 0, 1)              # [p, kc, rb, m]
            .reshape(128, KC * BS)
        )
        for a in range(NB)
    ]

    in_maps = []
    for c in range(NCORES):
        cells = core_cells(c)
        xa = np.concatenate([blkA[a] for a, b in cells], axis=0)
        xb = np.concatenate([blkT[b] for a, b in cells], axis=0)
        yc = np.concatenate(
            [np.broadcast_to(ybf[b * BS : (b + 1) * BS][None, :], (128, BS)) for a, b in cells],
            axis=0,
        )
        yo = np.concatenate(
            [ybf[a * BS : (a + 1) * BS].reshape(RBC, 128).T for a, b in cells], axis=1
        )
        in_maps.append(
            {
                "xa": np.ascontiguousarray(xa),
                "xb": np.ascontiguousarray(xb),
                "yc": np.ascontiguousarray(yc),
                "yo": np.ascontiguousarray(yo),
            }
        )
    return in_maps


def _assemble(results):
    """Combine per-core row/col partials into the final scalar loss."""
    sum_all = np.zeros(N, dtype=np.float64)
    sum_same = np.zeros(N, dtype=np.float64)
    for c in range(NCORES):
        r = results[c]
        cells = core_cells(c)
        ra = r["rows_all"].astype(np.float64)    # [128, NCELL*RBC]
        rs = r["rows_same"].astype(np.float64)
        ca = r["cols_all"].astype(np.float64).reshape(-1)   # [NCELL*BS]
        cs = r["cols_same"].astype(np.float64).reshape(-1)
        for k, (a, b) in enumerate(cells):
            for rb in range(RBC):
                rows = a * BS + rb * 128 + np.arange(128)
                sum_all[rows] += ra[:, k * RBC + rb]
                sum_same[rows] += rs[:, k * RBC + rb]
            if a != b:
                cols = b * BS + np.arange(BS)
                sum_all[cols] += ca[k * BS : (k + 1) * BS]
                sum_same[cols] += cs[k * BS : (k + 1) * BS]
    loss = np.log(sum_all) - np.log(sum_same)
    return np.float32(loss.mean())


def run(x, y, trace=False, **spmd_kwargs):
    nc = _get_nc()
    in_maps = _prep_inputs(x, y)
    res = run_bass_kernel_spmd(
        nc, in_maps, core_ids=list(range(NCORES)), trace=trace, **spmd_kwargs
    )
    return _assemble(res.results), res


def kernel(x, y, fp_v=None, **_ignored):
    val, _ = run(x, y, trace=False)
    return np.asarray(val, dtype=np.float32)



# revision 3
# speedup vs baseline: 3.2704x; 3.2704x over previous
"""fp8-DoubleRow contrastive-loss kernel for 8 NeuronCores.

s = xn @ xn.T is symmetric: only the 136 unordered band pairs {a, b} of a
16x16 grid of 512x512 cells are computed, via a star decomposition that is
CORE-UNIFORM in slot space: core c stores band (c+s) mod 16 in SBUF slot s
and runs two "stars":
  star 0 (a-slot 0): diag cell (0,0) + off-diag cells (0, s) s=1..8
  star 1 (a-slot 8): diag cell (8,8) + off-diag cells (8, s) s=9..15
Every unordered pair {i, j} with circular distance d=(j-i) mod 16 in 1..8 is
covered exactly once (by v=i if d<=7 or d=8 with i<8), 17 cells per core.

All 16 bands live in SBUF as fp8 (8 KB/partition each, 128 KB total), loaded
once by DMA (~47 us) and reused by every matmul; operands never re-stream.

Math per off-diag cell, with xq = round_fp8(xn * 256):
  PE : G = xq_a @ xq_b.T accumulated over 8 DoubleRow k-pairs (K=2048)
  ACT: e = exp(G / (256^2 tau)) -> fp8, fused f32 row-sum accum
  DVE: em = (yc == yo) * e -> fp8, fused f32 row-sum accum
  PE : colsum_e = ones.T @ e, colsum_em = ones.T @ em (DoubleRow over rb
       pairs), copied out via ACT/DVE then DMA.
Diag cells use a bf16 e (exp(10) overflows fp8) and skip colsums.
Off-diag cells are processed in PAIRS sharing one [128,1024] PSUM tile and
one ACT/DVE op, halving per-op overheads. Colsum matmuls for group g are
emitted after the main matmuls of group g+1 so the PE stream never stalls
on ACT/DVE results (keeps the tensor engine p-state at full clock).

Host: normalize + quantize + band layout; final assembly sums row/col
partials per band in f64 and takes mean(log(sum_all) - log(sum_same)).
"""

import sys

import numpy as np
import ml_dtypes

if "/opt/trn_rl_repo" not in sys.path:
    sys.path.insert(0, "/opt/trn_rl_repo")

import concourse.bass as bass  # noqa: F401  (bass types via tile/bacc)
import concourse.tile as tile
from concourse import bacc, mybir
from concourse.bass_utils import run_bass_kernel_spmd

TAU = 0.1
N, D = 8192, 2048
NCORES = 8
NB = 16                    # 512-row bands
BS = N // NB               # 512
KC = D // 128              # 16 contraction chunks
KP = KC // 2               # 8 DoubleRow k-pairs
RBC = BS // 128            # 4 row-blocks per band
QS = 256.0                 # fp8 quantization scale on xn
ASCALE = 1.0 / (QS * QS * TAU)
BF16 = mybir.dt.bfloat16
F32 = mybir.dt.float32
FP8 = mybir.dt.float8e4
DR = mybir.MatmulPerfMode.DoubleRow
NP_FP8 = ml_dtypes.float8_e4m3
NP_BF16 = ml_dtypes.bfloat16

# (a_slot, kind, b_slots); two stars, cells paired so ACT/DVE work on
# [128, 1024] tiles where possible.
GROUPS = [
    (0, "diag", (0,)),
    (0, "pair", (1, 2)),
    (0, "pair", (3, 4)),
    (0, "pair", (5, 6)),
    (0, "pair", (7, 8)),
    (8, "diag", (8,)),
    (8, "pair", (9, 10)),
    (8, "pair", (11, 12)),
    (8, "pair", (13, 14)),
    (8, "single", (15,)),
]
NG = len(GROUPS)           # 10 -> 40 row-accum slots
NOFF = 15                  # off-diag cells; cols slot for b_slot s is s-1

# Band/label DMAs interleaved with compute so colsum output DMAs are not
# starved behind one giant input stream, and band s arrives just before the
# first group that reads it.
DMA_PLAN = {
    0: [("band", 0), ("band", 1), ("band", 2)],
    1: [("yo",), ("ycq", 0), ("band", 3), ("band", 4)],
    2: [("ycq", 1), ("band", 5), ("band", 6)],
    3: [("band", 7), ("band", 8)],
    4: [("ycq", 2), ("band", 9), ("band", 10)],
    5: [("band", 11), ("band", 12)],
    6: [("ycq", 3), ("band", 13), ("band", 14)],
    7: [("band", 15)],
}


def build_bass():
    nc = bacc.Bacc(None, target_bir_lowering=False)

    bands_d = nc.dram_tensor("bands", [NB * 128, KC, BS], FP8, kind="ExternalInput")
    yc_d = nc.dram_tensor("yc", [128, N], BF16, kind="ExternalInput")
    yo_d = nc.dram_tensor("yo", [128, 8], BF16, kind="ExternalInput")
    rows_all = nc.dram_tensor("rows_all", [128, NG * RBC], F32, kind="ExternalOutput")
    rows_same = nc.dram_tensor("rows_same", [128, NG * RBC], F32, kind="ExternalOutput")
    cols_all = nc.dram_tensor("cols_all", [1, NOFF * BS], F32, kind="ExternalOutput")
    cols_same = nc.dram_tensor("cols_same", [1, NOFF * BS], F32, kind="ExternalOutput")

    with (
        tile.TileContext(nc) as tc,
        tc.tile_pool(name="res", bufs=1) as res,
        tc.tile_pool(name="ep", bufs=2) as ep,
        tc.tile_pool(name="emp", bufs=2) as emp,
        tc.tile_pool(name="dgp", bufs=2) as dgp,
        tc.tile_pool(name="csp", bufs=8) as csp,
        tc.tile_pool(name="pp", bufs=2, space="PSUM") as pp,
        tc.tile_pool(name="cpp", bufs=4, space="PSUM") as cpp,
    ):
        band_t = [res.tile([128, KC, BS], FP8, name=f"band{s}") for s in range(NB)]
        yc_t = res.tile([128, N], BF16)
        yo_t = res.tile([128, 8], BF16)
        # DoubleRow weights need a 3D AP [K, 2, M] with pair-step % 16 B == 0.
        ones_t = res.tile([128, 2, 16], FP8)
        nc.vector.memset(ones_t[:], 1.0)
        stage_all = res.tile([128, NG * RBC], F32)
        stage_same = res.tile([128, NG * RBC], F32)

        def emit_dma(item):
            if item[0] == "band":
                s = item[1]
                nc.sync.dma_start(
                    out=band_t[s][:], in_=bands_d[s * 128 : (s + 1) * 128, :, :]
                )
            elif item[0] == "yo":
                nc.sync.dma_start(out=yo_t[:], in_=yo_d[:])
            else:
                q = item[1]
                nc.sync.dma_start(
                    out=yc_t[:, q * 4 * BS : (q + 1) * 4 * BS],
                    in_=yc_d[:, q * 4 * BS : (q + 1) * 4 * BS],
                )

        pending_cols = None
        for g, (a_slot, kind, b_slots) in enumerate(GROUPS):
            for item in DMA_PLAN.get(g, ()):
                emit_dma(item)

            diag = kind == "diag"
            w = len(b_slots) * BS
            if not diag:
                e_t = ep.tile([128, RBC, 1024], FP8, name="e_t")
                em_t = emp.tile([128, RBC, 1024], FP8, name="em_t")
            for rb in range(RBC):
                ps = pp.tile([128, 1024], F32, name="ps")
                for h, b in enumerate(b_slots):
                    for t in range(KP):
                        nc.tensor.matmul(
                            ps[:, h * BS : (h + 1) * BS],
                            band_t[a_slot][:, 2 * t : 2 * t + 2, rb * 128 : (rb + 1) * 128],
                            band_t[b][:, 2 * t : 2 * t + 2, :],
                            start=(t == 0),
                            stop=(t == KP - 1),
                            perf_mode=DR,
                        )
                slot = g * RBC + rb
                ycol = (0 if a_slot == 0 else 4) + rb
                yc_in = yc_t[:, b_slots[0] * BS : b_slots[0] * BS + w]
                if diag:
                    e_dg = dgp.tile([128, BS], BF16, name="e_dg")
                    nc.scalar.activation(
                        out=e_dg[:],
                        in_=ps[:, 0:BS],
                        func=mybir.ActivationFunctionType.Exp,
                        scale=ASCALE,
                        accum_out=stage_all[:, slot : slot + 1],
                    )
                    em_dg = dgp.tile([128, BS], BF16, name="em_dg")
                    nc.vector.scalar_tensor_tensor(
                        out=em_dg[:],
                        in0=yc_in,
                        scalar=yo_t[:, ycol : ycol + 1],
                        in1=e_dg[:],
                        op0=mybir.AluOpType.is_equal,
                        op1=mybir.AluOpType.mult,
                        accum_out=stage_same[:, slot : slot + 1],
                    )
                else:
                    nc.scalar.activation(
                        out=e_t[:, rb, 0:w],
                        in_=ps[:, 0:w],
                        func=mybir.ActivationFunctionType.Exp,
                        scale=ASCALE,
                        accum_out=stage_all[:, slot : slot + 1],
                    )
                    nc.vector.scalar_tensor_tensor(
                        out=em_t[:, rb, 0:w],
                        in0=yc_in,
                        scalar=yo_t[:, ycol : ycol + 1],
                        in1=e_t[:, rb, 0:w],
                        op0=mybir.AluOpType.is_equal,
                        op1=mybir.AluOpType.mult,
                        accum_out=stage_same[:, slot : slot + 1],
                    )

            # Colsums of the PREVIOUS off-diag group: by now its e/em tiles
            # are ready, so the PE never waits on ACT/DVE.
            if pending_cols is not None:
                pending_cols()
                pending_cols = None

            if not diag:

                def make_cols(e_t=e_t, em_t=em_t, b_slots=b_slots):
                    def emit():
                        for h, b in enumerate(b_slots):
                            cps_e = cpp.tile([1, BS], F32, name="cps_e", tag="col")
                            for t2 in range(2):
                                nc.tensor.matmul(
                                    cps_e[:],
                                    ones_t[:, 0:2, 0:1],
                                    e_t[:, 2 * t2 : 2 * t2 + 2, h * BS : (h + 1) * BS],
                                    start=(t2 == 0),
                                    stop=(t2 == 1),
                                    perf_mode=DR,
                                )
                            cps_m = cpp.tile([1, BS], F32, name="cps_m", tag="col")
                            for t2 in range(2):
                                nc.tensor.matmul(
                                    cps_m[:],
                                    ones_t[:, 0:2, 0:1],
                                    em_t[:, 2 * t2 : 2 * t2 + 2, h * BS : (h + 1) * BS],
                                    start=(t2 == 0),
                                    stop=(t2 == 1),
                                    perf_mode=DR,
                                )
                            cidx = b - 1
                            ce = csp.tile([1, BS], F32, name="cse", tag="cs")
                            nc.scalar.copy(out=ce[:], in_=cps_e[:])
                            nc.sync.dma_start(
                                out=cols_all[:, cidx * BS : (cidx + 1) * BS], in_=ce[:]
                            )
                            cm = csp.tile([1, BS], F32, name="csm", tag="cs")
                            nc.vector.tensor_copy(out=cm[:], in_=cps_m[:])
                            nc.sync.dma_start(
                                out=cols_same[:, cidx * BS : (cidx + 1) * BS], in_=cm[:]
                            )

                    return emit

                pending_cols = make_cols()

        if pending_cols is not None:
            pending_cols()

        nc.sync.dma_start(out=rows_all[:], in_=stage_all[:])
        nc.sync.dma_start(out=rows_same[:], in_=stage_same[:])

    nc.compile()
    return nc


_CACHE: dict = {}


def _get_nc():
    if "nc" not in _CACHE:
        _CACHE["nc"] = build_bass()
    return _CACHE["nc"]


def _prep_inputs(x, y):
    x = np.ascontiguousarray(np.asarray(x, dtype=np.float32))
    y = np.asarray(y).astype(np.int64)
    xn = x / np.linalg.norm(x, axis=1, keepdims=True)
    # Renormalize in the quantized domain: rescale each row so its QUANTIZED
    # norm is exactly QS, killing the systematic s_ii bias from fp8 rounding.
    xq = (xn * QS).astype(NP_FP8)
    nrm = np.sqrt((xq.astype(np.float32) ** 2).sum(1, keepdims=True)) / QS
    xq = ((xn * QS) / nrm).astype(NP_FP8)
    ybf = y.astype(NP_BF16)

    # band[b][p, kc, jj] = xq[b*BS + jj, kc*128 + p]
    bandmat = [
        np.ascontiguousarray(xq[b * BS : (b + 1) * BS].reshape(BS, KC, 128).transpose(2, 1, 0))
        for b in range(NB)
    ]

    in_maps = []
    for c in range(NCORES):
        perm = [(c + s) % NB for s in range(NB)]
        bands = np.concatenate([bandmat[b] for b in perm], axis=0)
        ycat = np.concatenate([ybf[b * BS : (b + 1) * BS] for b in perm])
        yc = np.ascontiguousarray(np.broadcast_to(ycat[None, :], (128, N)))
        yo = np.ascontiguousarray(
            np.concatenate(
                [
                    ybf[perm[a] * BS : (perm[a] + 1) * BS].reshape(RBC, 128).T
                    for a in (0, 8)
                ],
                axis=1,
            )
        )
        in_maps.append({"bands": bands, "yc": yc, "yo": yo})
    return in_maps


def _assemble(results):
    sum_all = np.zeros(N, dtype=np.float64)
    sum_same = np.zeros(N, dtype=np.float64)
    for c in range(NCORES):
        r = results[c]
        perm = [(c + s) % NB for s in range(NB)]
        ra = r["rows_all"].astype(np.float64)
        rs = r["rows_same"].astype(np.float64)
        ca = r["cols_all"].astype(np.float64).reshape(-1)
        cs_ = r["cols_same"].astype(np.float64).reshape(-1)
        for g, (a_slot, kind, b_slots) in enumerate(GROUPS):
            ab = perm[a_slot]
            for rb in range(RBC):
                rows = ab * BS + rb * 128 + np.arange(128)
                sum_all[rows] += ra[:, g * RBC + rb]
                sum_same[rows] += rs[:, g * RBC + rb]
            if kind != "diag":
                for b in b_slots:
                    cols = perm[b] * BS + np.arange(BS)
                    cidx = b - 1
                    sum_all[cols] += ca[cidx * BS : (cidx + 1) * BS]
                    sum_same[cols] += cs_[cidx * BS : (cidx + 1) * BS]
    loss = np.log(sum_all) - np.log(sum_same)
    return np.float32(loss.mean())


def run(x, y, trace=False, **spmd_kwargs):
    nc = _get_nc()
    in_maps = _prep_inputs(x, y)
    res = run_bass_kernel_spmd(
        nc, in_maps, core_ids=list(range(NCORES)), trace=trace, **spmd_kwargs
    )
    return _assemble(res.results), res


def kernel(x, y, fp_v=None, **_ignored):
    val, _ = run(x, y, trace=False)
    return np.asarray(val, dtype=np.float32)


# revision 5
# speedup vs baseline: 3.3969x; 1.0387x over previous
"""fp8-DoubleRow contrastive-loss kernel for 8 NeuronCores.

s = xn @ xn.T is symmetric: only the 136 unordered band pairs {a, b} of a
16x16 grid of 512x512 cells are computed, via a star decomposition that is
CORE-UNIFORM in slot space: core c stores band (c+s) mod 16 in SBUF slot s
and runs two "stars":
  star 0 (a-slot 0): diag cell (0,0) + off-diag cells (0, s) s=1..8
  star 1 (a-slot 8): diag cell (8,8) + off-diag cells (8, s) s=9..15
Every unordered pair {i, j} with circular distance d=(j-i) mod 16 in 1..8 is
covered exactly once (by v=i if d<=7 or d=8 with i<8), 17 cells per core.

All 16 bands live in SBUF as fp8 (8 KB/partition each, 128 KB total), loaded
once by DMA (~47 us) and reused by every matmul; operands never re-stream.

Math per off-diag cell, with xq = round_fp8(xn * 256):
  PE : G = xq_a @ xq_b.T accumulated over 8 DoubleRow k-pairs (K=2048)
  ACT: e = exp(G / (256^2 tau)) -> fp8, fused f32 row-sum accum
  DVE: em = (yc == yo) * e -> fp8, fused f32 row-sum accum
  PE : colsum_e = ones.T @ e, colsum_em = ones.T @ em (DoubleRow over rb
       pairs), copied out via ACT/DVE then DMA.
Diag cells use a bf16 e (exp(10) overflows fp8) and skip colsums.
Off-diag cells are processed in PAIRS sharing one [128,1024] PSUM tile and
one ACT/DVE op, halving per-op overheads. Colsum matmuls for group g are
emitted after the main matmuls of group g+1 so the PE stream never stalls
on ACT/DVE results (keeps the tensor engine p-state at full clock).

Host: normalize + quantize + band layout; final assembly sums row/col
partials per band in f64 and takes mean(log(sum_all) - log(sum_same)).
"""

import sys

import numpy as np
import ml_dtypes

if "/opt/trn_rl_repo" not in sys.path:
    sys.path.insert(0, "/opt/trn_rl_repo")

import concourse.bass as bass  # noqa: F401  (bass types via tile/bacc)
import concourse.tile as tile
from concourse import bacc, mybir
from concourse.bass_utils import run_bass_kernel_spmd

TAU = 0.1
N, D = 8192, 2048
NCORES = 8
NB = 16                    # 512-row bands
BS = N // NB               # 512
KC = D // 128              # 16 contraction chunks
KP = KC // 2               # 8 DoubleRow k-pairs
RBC = BS // 128            # 4 row-blocks per band
QS = 256.0                 # fp8 quantization scale on xn
ASCALE = 1.0 / (QS * QS * TAU)
BF16 = mybir.dt.bfloat16
F32 = mybir.dt.float32
FP8 = mybir.dt.float8e4
DR = mybir.MatmulPerfMode.DoubleRow
NP_FP8 = ml_dtypes.float8_e4m3
NP_BF16 = ml_dtypes.bfloat16

# (a_slot, kind, b_slots); two stars, cells paired so ACT/DVE work on
# [128, 1024] tiles where possible.
GROUPS = [
    (0, "diag", (0,)),
    (0, "pair", (1, 2)),
    (0, "pair", (3, 4)),
    (0, "pair", (5, 6)),
    (0, "pair", (7, 8)),
    (8, "diag", (8,)),
    (8, "pair", (9, 10)),
    (8, "pair", (11, 12)),
    (8, "pair", (13, 14)),
    (8, "single", (15,)),
]
NG = len(GROUPS)           # 10 -> 40 row-accum slots
NOFF = 15                  # off-diag cells; cols slot for b_slot s is s-1

# Band/label DMAs interleaved with compute so colsum output DMAs are not
# starved behind one giant input stream, and band s arrives just before the
# first group that reads it.
DMA_PLAN = {
    0: [("band", 0), ("band", 1), ("band", 2)],
    1: [("yo",), ("ycq", 0), ("band", 3), ("band", 4)],
    2: [("ycq", 1), ("band", 5), ("band", 6)],
    3: [("band", 7), ("band", 8)],
    4: [("ycq", 2), ("band", 9), ("band", 10)],
    5: [("band", 11), ("band", 12)],
    6: [("ycq", 3), ("band", 13), ("band", 14)],
    7: [("band", 15)],
}


def build_bass():
    nc = bacc.Bacc(None, target_bir_lowering=False)

    bands_d = nc.dram_tensor("bands", [NB * 128, KC, BS], FP8, kind="ExternalInput")
    yc_d = nc.dram_tensor("yc", [128, N], BF16, kind="ExternalInput")
    yo_d = nc.dram_tensor("yo", [128, 8], BF16, kind="ExternalInput")
    rows_all = nc.dram_tensor("rows_all", [128, NG * RBC], F32, kind="ExternalOutput")
    rows_same = nc.dram_tensor("rows_same", [128, NG * RBC], F32, kind="ExternalOutput")
    cols_all = nc.dram_tensor("cols_all", [1, NOFF * BS], F32, kind="ExternalOutput")
    cols_same = nc.dram_tensor("cols_same", [1, NOFF * BS], F32, kind="ExternalOutput")

    with (
        tile.TileContext(nc) as tc,
        tc.tile_pool(name="res", bufs=1) as res,
        tc.tile_pool(name="ep", bufs=2) as ep,
        tc.tile_pool(name="emp", bufs=2) as emp,
        tc.tile_pool(name="dgp", bufs=2) as dgp,
        tc.tile_pool(name="csp", bufs=8) as csp,
        tc.tile_pool(name="pp", bufs=2, space="PSUM") as pp,
        tc.tile_pool(name="cpp", bufs=4, space="PSUM") as cpp,
    ):
        band_t = [res.tile([128, KC, BS], FP8, name=f"band{s}") for s in range(NB)]
        yc_t = res.tile([128, N], BF16)
        yo_t = res.tile([128, 8], BF16)
        # DoubleRow weights need a 3D AP [K, 2, M] with pair-step % 16 B == 0.
        ones_t = res.tile([128, 2, 16], FP8)
        nc.vector.memset(ones_t[:], 1.0)
        stage_all = res.tile([128, NG * RBC], F32)
        stage_same = res.tile([128, NG * RBC], F32)

        def emit_dma(item):
            if item[0] == "band":
                # kc-halves: the first DoubleRow matmuls (t=0..3) can start
                # as soon as the first half lands (subtile deps).
                s = item[1]
                for hh in range(2):
                    nc.sync.dma_start(
                        out=band_t[s][:, hh * 8 : (hh + 1) * 8, :],
                        in_=bands_d[s * 128 : (s + 1) * 128, hh * 8 : (hh + 1) * 8, :],
                    )
            elif item[0] == "yo":
                nc.sync.dma_start(out=yo_t[:], in_=yo_d[:])
            else:
                q = item[1]
                nc.sync.dma_start(
                    out=yc_t[:, q * 4 * BS : (q + 1) * 4 * BS],
                    in_=yc_d[:, q * 4 * BS : (q + 1) * 4 * BS],
                )

        pending_cols = None
        for g, (a_slot, kind, b_slots) in enumerate(GROUPS):
            for item in DMA_PLAN.get(g, ()):
                emit_dma(item)

            diag = kind == "diag"
            w = len(b_slots) * BS
            if not diag:
                e_t = ep.tile([128, RBC, 1024], FP8, name="e_t")
                em_t = emp.tile([128, RBC, 1024], FP8, name="em_t")
            for rb in range(RBC):
                ps = pp.tile([128, 1024], F32, name="ps")
                for h, b in enumerate(b_slots):
                    for t in range(KP):
                        nc.tensor.matmul(
                            ps[:, h * BS : (h + 1) * BS],
                            band_t[a_slot][:, 2 * t : 2 * t + 2, rb * 128 : (rb + 1) * 128],
                            band_t[b][:, 2 * t : 2 * t + 2, :],
                            start=(t == 0),
                            stop=(t == KP - 1),
                            perf_mode=DR,
                        )
                slot = g * RBC + rb
                ycol = (0 if a_slot == 0 else 4) + rb
                yc_in = yc_t[:, b_slots[0] * BS : b_slots[0] * BS + w]
                if diag:
                    e_dg = dgp.tile([128, BS], BF16, name="e_dg")
                    nc.scalar.activation(
                        out=e_dg[:],
                        in_=ps[:, 0:BS],
                        func=mybir.ActivationFunctionType.Exp,
                        scale=ASCALE,
                        accum_out=stage_all[:, slot : slot + 1],
                    )
                    em_dg = dgp.tile([128, BS], BF16, name="em_dg")
                    nc.vector.scalar_tensor_tensor(
                        out=em_dg[:],
                        in0=yc_in,
                        scalar=yo_t[:, ycol : ycol + 1],
                        in1=e_dg[:],
                        op0=mybir.AluOpType.is_equal,
                        op1=mybir.AluOpType.mult,
                        accum_out=stage_same[:, slot : slot + 1],
                    )
                else:
                    nc.scalar.activation(
                        out=e_t[:, rb, 0:w],
                        in_=ps[:, 0:w],
                        func=mybir.ActivationFunctionType.Exp,
                        scale=ASCALE,
                        accum_out=stage_all[:, slot : slot + 1],
                    )
                    nc.vector.scalar_tensor_tensor(
                        out=em_t[:, rb, 0:w],
                        in0=yc_in,
                        scalar=yo_t[:, ycol : ycol + 1],
                        in1=e_t[:, rb, 0:w],
                        op0=mybir.AluOpType.is_equal,
                        op1=mybir.AluOpType.mult,
                        accum_out=stage_same[:, slot : slot + 1],
                    )

            # Colsums of the PREVIOUS off-diag group: by now its e/em tiles
            # are ready, so the PE never waits on ACT/DVE.
            if pending_cols is not None:
                pending_cols()
                pending_cols = None

            if not diag:

                def make_cols(e_t=e_t, em_t=em_t, b_slots=b_slots):
                    def emit():
                        for h, b in enumerate(b_slots):
                            cps_e = cpp.tile([1, BS], F32, name="cps_e", tag="col")
                            for t2 in range(2):
                                nc.tensor.matmul(
                                    cps_e[:],
                                    ones_t[:, 0:2, 0:1],
                                    e_t[:, 2 * t2 : 2 * t2 + 2, h * BS : (h + 1) * BS],
                                    start=(t2 == 0),
                                    stop=(t2 == 1),
                                    perf_mode=DR,
                                )
                            cps_m = cpp.tile([1, BS], F32, name="cps_m", tag="col")
                            for t2 in range(2):
                                nc.tensor.matmul(
                                    cps_m[:],
                                    ones_t[:, 0:2, 0:1],
                                    em_t[:, 2 * t2 : 2 * t2 + 2, h * BS : (h + 1) * BS],
                                    start=(t2 == 0),
                                    stop=(t2 == 1),
                                    perf_mode=DR,
                                )
                            cidx = b - 1
                            # Both colsum copies on DVE: ACT must stay under
                            # PE's cadence or the psum ring stalls the PE.
                            ce = csp.tile([1, BS], F32, name="cse", tag="cs")
                            nc.vector.tensor_copy(out=ce[:], in_=cps_e[:])
                            nc.sync.dma_start(
                                out=cols_all[:, cidx * BS : (cidx + 1) * BS], in_=ce[:]
                            )
                            cm = csp.tile([1, BS], F32, name="csm", tag="cs")
                            nc.vector.tensor_copy(out=cm[:], in_=cps_m[:])
                            nc.sync.dma_start(
                                out=cols_same[:, cidx * BS : (cidx + 1) * BS], in_=cm[:]
                            )

                    return emit

                pending_cols = make_cols()

        if pending_cols is not None:
            pending_cols()

        nc.sync.dma_start(out=rows_all[:], in_=stage_all[:])
        nc.sync.dma_start(out=rows_same[:], in_=stage_same[:])

    nc.compile()
    return nc


_CACHE: dict = {}


def _get_nc():
    if "nc" not in _CACHE:
        _CACHE["nc"] = build_bass()
    return _CACHE["nc"]


def _prep_inputs(x, y):
    x = np.ascontiguousarray(np.asarray(x, dtype=np.float32))
    y = np.asarray(y).astype(np.int64)
    xn = x / np.linalg.norm(x, axis=1, keepdims=True)
    # Renormalize in the quantized domain: rescale each row so its QUANTIZED
    # norm is exactly QS, killing the systematic s_ii bias from fp8 rounding.
    xq = (xn * QS).astype(NP_FP8)
    nrm = np.sqrt((xq.astype(np.float32) ** 2).sum(1, keepdims=True)) / QS
    xq = ((xn * QS) / nrm).astype(NP_FP8)
    ybf = y.astype(NP_BF16)

    # band[b][p, kc, jj] = xq[b*BS + jj, kc*128 + p]
    bandmat = [
        np.ascontiguousarray(xq[b * BS : (b + 1) * BS].reshape(BS, KC, 128).transpose(2, 1, 0))
        for b in range(NB)
    ]

    in_maps = []
    for c in range(NCORES):
        perm = [(c + s) % NB for s in range(NB)]
        bands = np.concatenate([bandmat[b] for b in perm], axis=0)
        ycat = np.concatenate([ybf[b * BS : (b + 1) * BS] for b in perm])
        yc = np.ascontiguousarray(np.broadcast_to(ycat[None, :], (128, N)))
        yo = np.ascontiguousarray(
            np.concatenate(
                [
                    ybf[perm[a] * BS : (perm[a] + 1) * BS].reshape(RBC, 128).T
                    for a in (0, 8)
                ],
                axis=1,
            )
        )
        in_maps.append({"bands": bands, "yc": yc, "yo": yo})
    return in_maps


def _assemble(results):
    sum_all = np.zeros(N, dtype=np.float64)
    sum_same = np.zeros(N, dtype=np.float64)
    for c in range(NCORES):
        r = results[c]
        perm = [(c + s) % NB for s in range(NB)]
        ra = r["rows_all"].astype(np.float64)
        rs = r["rows_same"].astype(np.float64)
        ca = r["cols_all"].astype(np.float64).reshape(-1)
        cs_ = r["cols_same"].astype(np.float64).reshape(-1)
        for g, (a_slot, kind, b_slots) in enumerate(GROUPS):
            ab = perm[a_slot]
            for rb in range(RBC):
                rows = ab * BS + rb * 128 + np.arange(128)
                sum_all[rows] += ra[:, g * RBC + rb]
                sum_same[rows] += rs[:, g * RBC + rb]
            if kind != "diag":
                for b in b_slots:
                    cols = perm[b] * BS + np.arange(BS)
                    cidx = b - 1
                    sum_all[cols] += ca[cidx * BS : (cidx + 1) * BS]
                    sum_same[cols] += cs_[cidx * BS : (cidx + 1) * BS]
    loss = np.log(sum_all) - np.log(sum_same)
    return np.float32(loss.mean())


def run(x, y, trace=False, **spmd_kwargs):
    nc = _get_nc()
    in_maps = _prep_inputs(x, y)
    res = run_bass_kernel_spmd(
        nc, in_maps, core_ids=list(range(NCORES)), trace=trace, **spmd_kwargs
    )
    return _assemble(res.results), res


def kernel(x, y, fp_v=None, **_ignored):
    val, _ = run(x, y, trace=False)
    return np.asarray(val, dtype=np.float32)
